# revision 23
# baseline (speedup 1.0000x reference)
"""Trainium2 Bass kernel for nn_MinCostMatcher (focal-cls + L1 + GIoU matcher).

Per core = one batch element (data-parallel over b=8).

Cost (per m over n): total/2 = cls + 2.5*sum_k|D_k| + 1 - iou + gterm.
Dropping per-m constants, argmin_n total == argmax_n NF with
  NF = -cls - 5*(u_y+u_x) - 2.5*sp[n] + iou - gterm
where u_y = relu(Dy1)+relu(Dy2) etc (the |.| and min/max identities).

Device work:
  - stage 1: X[c,n] focal table = 0.75 p^2 ln(q+eps) - 0.25 q^2 ln(p+eps),
    computed on a flat [128, 10240] view of the class-major table, written
    to DRAM; then an indirect-DMA gather pulls row label[m] into an SBUF
    [M, N] fp32 matrix Gm (exact cls term, no one-hot matmul needed).
  - pairwise: PE builds rank-2 cross-difference matrices from host-prepped
    fp16 hi/lo weights: Dp1=[Dy1|Dx1], Dp2=[Dy2|Dx2], K=[Ky|Kx] with
    Ky=ty2-py1 (so i_h pre-clamp = K - u, no per-m scalar), U0=pa+ta,
    L0=-2.5*sp.
  - reciprocals via Exp(-Ln(x)) on the scalar engine (ACT spline, ~1e-6
    rel err, margins are >=1.2e-3).
  - gterm = max(min(1 - union/eden, enc*1e30), 0) which matches the
    reference's where(enclose>0, ...) exactly, including degenerate boxes.
  - argmax: per-4096 super-chunk max8/max_index8 + running top-1 merge.

Host prep (input marshaling only): class-major reshape of cls_pred, labels
and cls_id from cls_true (argmax of an exact one-hot), fp16 hi/lo splits of
per-m/per-n scalars, the PE weight tiles, and per-n rhs rows.
"""

import numpy as np
from contextlib import ExitStack

import concourse.bass as bass
import concourse.bacc as bacc
import concourse.tile as tile
from concourse import mybir
from concourse import bass_utils

F32 = mybir.dt.float32
F16 = mybir.dt.float16
I32 = mybir.dt.int32
U32 = mybir.dt.uint32
Alu = mybir.AluOpType
Act = mybir.ActivationFunctionType

B = 8
N = 16384
C = 80
M = 100
EPS = 1e-8
NT = 512             # chunk width (one PSUM bank)
SUP = 4096           # super-chunk for rhs loads + argmax
NSUP = N // SUP      # 4
CPS = SUP // NT      # 8 chunks per super
QF = 1280            # stage-1 flat free chunk
QCH = (N * C) // (128 * QF)  # 8
NEG_INF = -3.0e38
BIG = 1.0e30


def emit_kernel(nc: bass.Bass, t: dict):
    cp = t["cp"].ap()        # (128, 10240) f32  class-major flat table input
    r1 = t["r1"].ap()        # (16, N) f16  [py1h,py1l,1,1, px1h,px1l,1,1, py2h,py2l,1,1, px2h,px2l,1,1]
    r2 = t["r2"].ap()        # (6, N)  f16  [pah,pal,1,1, sph',spl']  (sp' = -2.5*sp)
    la = t["la"].ap()        # (128, M) f16 lhsT: Dy1@0, Dx1@32, Dy2@64, Dx2@96
    lb = t["lb"].ap()        # (128, M) f16 lhsT: Ky@0, Kx@32, Ly@64, Lx@96
    lc = t["lc"].ap()        # (6, M)   f16 lhsT: U0@0
    hw = t["hw"].ap()        # (M, 2) f32  [ht, wt]
    spf = t["srow"].ap()     # (128, 10240) f32
    lab = t["lab"].ap()      # (M, 1) i32  labels
    meta = t["meta"].ap()    # (M, 2) i32  [bidx, cls_id]
    xtp = t["xtp"].ap()      # (80, N) f32 scratch: X table, class-major
    out = t["out"].ap()      # (M, 3) i32

    xtp_flat = xtp.rearrange("c n -> (c n)").rearrange("(p f) -> p f", p=128)

    with tile.TileContext(nc) as tc, ExitStack() as ctx:
        singles = ctx.enter_context(tc.tile_pool(name="singles", bufs=1))
        eps_col = singles.tile([128, 1], F32)
        nc.vector.memset(eps_col, EPS)

        # ---------------- stage 1: focal table -> DRAM (class-major) ------
        with tc.tile_pool(name="s1", bufs=3) as s1:
            for j in range(QCH):
                sl = slice(j * QF, (j + 1) * QF)
                pj = s1.tile([128, QF], F32, tag="pj")
                nc.gpsimd.dma_start(out=pj, in_=cp[:, sl])
                qj = s1.tile([128, QF], F32, tag="qj")
                nc.vector.tensor_scalar(qj, pj, 1.0, -1.0, Alu.subtract, Alu.mult)
                lnp = s1.tile([128, QF], F32, tag="lnp")
                nc.scalar.activation(lnp, pj, Act.Ln, bias=eps_col, scale=1.0)
                lnq = s1.tile([128, QF], F32, tag="lnq")
                nc.scalar.activation(lnq, qj, Act.Ln, bias=eps_col, scale=1.0)
                sqp = s1.tile([128, QF], F32, tag="sqp")
                nc.scalar.activation(sqp, pj, Act.Square)
                sqq = s1.tile([128, QF], F32, tag="sqq")
                nc.scalar.activation(sqq, qj, Act.Square)
                t2 = s1.tile([128, QF], F32, tag="t2")
                nc.vector.scalar_tensor_tensor(t2, sqp, -0.75, lnq, Alu.mult, Alu.mult)
                x2n = s1.tile([128, QF], F32, tag="x2n")
                nc.vector.scalar_tensor_tensor(x2n, sqq, 0.25, lnp, Alu.mult, Alu.mult)
                spj = s1.tile([128, QF], F32, tag="spj")
                nc.gpsimd.dma_start(out=spj, in_=spf[:, sl])
                xj = s1.tile([128, QF], F32, tag="qj")
                nc.gpsimd.tensor_tensor(xj, t2, x2n, Alu.add)
                xj2 = s1.tile([128, QF], F32, tag="pj")
                nc.vector.tensor_tensor(xj2, xj, spj, Alu.add)
                nc.sync.dma_start(out=xtp_flat[:, sl], in_=xj2)

        # ---------------- small per-m tensors ------------------------------
        lhsA = singles.tile([68, M], F16)
        nc.sync.dma_start(out=lhsA[0:4, :], in_=la[0:4, :])
        nc.sync.dma_start(out=lhsA[32:36, :], in_=la[32:36, :])
        nc.sync.dma_start(out=lhsA[64:68, :], in_=la[64:68, :])
        lhsA2 = singles.tile([68, M], F16)
        nc.sync.dma_start(out=lhsA2[64:68, :], in_=la[96:100, :])
        lhsB = singles.tile([68, M], F16)
        nc.sync.dma_start(out=lhsB[0:4, :], in_=lb[0:4, :])
        nc.sync.dma_start(out=lhsB[32:36, :], in_=lb[32:36, :])
        nc.sync.dma_start(out=lhsB[64:68, :], in_=lb[64:68, :])
        lhsB2 = singles.tile([68, M], F16)
        nc.sync.dma_start(out=lhsB2[64:68, :], in_=lb[96:100, :])
        lhsC = singles.tile([36, M], F16)
        nc.sync.dma_start(out=lhsC, in_=lc)
        hwt = singles.tile([M, 2], F32)
        nc.sync.dma_start(out=hwt, in_=hw)
        labt = singles.tile([M, 1], I32)
        nc.sync.dma_start(out=labt, in_=lab)
        metat = singles.tile([M, 2], I32)
        nc.sync.dma_start(out=metat, in_=meta)

        # ------- gather cls rows: Gm[m, :] = -X[label_m, :] - 2.5*sp[:] ----
        Gm = singles.tile([M, N], F32)
        nc.gpsimd.indirect_dma_start(
            out=Gm[:, :],
            out_offset=None,
            in_=xtp,
            in_offset=bass.IndirectOffsetOnAxis(ap=labt[:, 0:1], axis=0),
        )

        # ---------------- running argmax state -----------------------------
        bv = singles.tile([M, 1], F32)
        nc.vector.memset(bv, NEG_INF)
        bi = singles.tile([M, 1], U32)
        nc.vector.memset(bi, 0)

        # ---------------- pairwise main loop -------------------------------
        with tc.tile_pool(name="ps", bufs=1, space="PSUM") as ps, \
             tc.tile_pool(name="rhs", bufs=1) as rp, \
             tc.tile_pool(name="pw", bufs=1) as pw, \
             tc.tile_pool(name="sp2", bufs=1) as sp2, \
             tc.tile_pool(name="nf", bufs=2) as nfp, \
             tc.tile_pool(name="mg", bufs=2) as mg:
            for s in range(NSUP):
                ssl = slice(s * SUP, (s + 1) * SUP)
                R1c = rp.tile([68, SUP], F16, tag="r1c", bufs=2)
                nc.sync.dma_start(out=R1c[0:4, :], in_=r1[0:4, ssl])
                nc.sync.dma_start(out=R1c[32:36, :], in_=r1[4:8, ssl])
                nc.sync.dma_start(out=R1c[64:68, :], in_=r1[8:12, ssl])
                R2c = rp.tile([68, SUP], F16, tag="r2c", bufs=1)
                nc.sync.dma_start(out=R2c[0:4, :], in_=r2[0:4, ssl])
                nc.sync.dma_start(out=R2c[64:68, :], in_=r1[12:16, ssl])

                # per-super persistent buffers (all 2D, dense regions)
                # TAILB = [inter(0:1024) | union(1024:2048) | enc(2048:3072)]
                TAILB = sp2.tile([M, 3 * SUP], F32, tag="tailb", bufs=2)
                SRS = sp2.tile([M, SUP], F32, tag="srs", bufs=2)
                NF = nfp.tile([M, SUP], F32, tag="nf")

                # ---- front pass: matmuls, relus, geometry ----
                for jc in range(CPS):
                    cs = slice(jc * NT, (jc + 1) * NT)          # in super

                    D4 = ps.tile([M, 4 * NT], F32, tag="d4", bufs=1)
                    nc.tensor.matmul(D4[:, 0:NT], lhsA[0:4, :], R1c[0:4, cs],
                                     start=True, stop=True)
                    nc.tensor.matmul(D4[:, NT:2 * NT], lhsA[32:36, :],
                                     R1c[32:36, cs], start=True, stop=True)
                    nc.tensor.matmul(D4[:, 2 * NT:3 * NT], lhsA[64:68, :],
                                     R1c[64:68, cs], start=True, stop=True)
                    nc.tensor.matmul(D4[:, 3 * NT:4 * NT], lhsA2[64:68, :],
                                     R2c[64:68, cs], start=True, stop=True)
                    U0 = ps.tile([M, NT], F32, tag="u0", bufs=2)
                    nc.tensor.matmul(U0, lhsC[0:4, :], R2c[0:4, cs],
                                     start=True, stop=True)

                    rbuf = pw.tile([M, 4 * NT], F32, tag="rbuf", bufs=2)
                    nc.scalar.activation(rbuf[:, 0:2 * NT], D4[:, 0:2 * NT],
                                         Act.Relu)
                    nc.scalar.activation(rbuf[:, 2 * NT:4 * NT],
                                         D4[:, 2 * NT:4 * NT], Act.Relu)
                    u2 = pw.tile([M, 2 * NT], F32, tag="u2", bufs=2)
                    nc.gpsimd.tensor_tensor(u2, rbuf[:, 0:2 * NT],
                                            rbuf[:, 2 * NT:4 * NT], Alu.add)
                    # overwrite D4 with [Ky|Kx|Ly|Lx]
                    nc.tensor.matmul(D4[:, 0:NT], lhsB[0:4, :], R1c[0:4, cs],
                                     start=True, stop=True)
                    nc.tensor.matmul(D4[:, NT:2 * NT], lhsB[32:36, :],
                                     R1c[32:36, cs], start=True, stop=True)
                    nc.tensor.matmul(D4[:, 2 * NT:3 * NT], lhsB[64:68, :],
                                     R1c[64:68, cs], start=True, stop=True)
                    nc.tensor.matmul(D4[:, 3 * NT:4 * NT], lhsB2[64:68, :],
                                     R2c[64:68, cs], start=True, stop=True)
                    d2 = pw.tile([M, 2 * NT], F32, tag="d2", bufs=1)
                    nc.vector.tensor_tensor(d2, D4[:, 0:2 * NT], u2,
                                            Alu.subtract)
                    ihw = pw.tile([M, 2 * NT], F32, tag="ihw", bufs=1)
                    nc.scalar.activation(ihw, d2, Act.Relu)
                    e2 = pw.tile([M, 2 * NT], F32, tag="e2", bufs=1)
                    nc.vector.tensor_tensor(e2, u2, D4[:, 2 * NT:4 * NT],
                                            Alu.subtract)
                    nc.gpsimd.tensor_tensor(TAILB[:, jc * NT:(jc + 1) * NT],
                                            ihw[:, 0:NT], ihw[:, NT:2 * NT],
                                            Alu.mult)
                    nc.vector.tensor_tensor(
                        TAILB[:, 2 * SUP + jc * NT:2 * SUP + (jc + 1) * NT],
                        e2[:, 0:NT], e2[:, NT:2 * NT], Alu.mult)
                    nc.vector.tensor_tensor(
                        TAILB[:, SUP + jc * NT:SUP + (jc + 1) * NT], U0,
                        TAILB[:, jc * NT:(jc + 1) * NT], Alu.subtract)
                    nc.gpsimd.tensor_tensor(SRS[:, jc * NT:(jc + 1) * NT],
                                            u2[:, 0:NT], u2[:, NT:2 * NT],
                                            Alu.add)

                # ---- batched tail over the whole super ----
                CLD = sp2.tile([M, 2 * SUP], F32, tag="cld", bufs=1)
                nc.vector.tensor_scalar(CLD, TAILB[:, SUP:3 * SUP], EPS, None,
                                        Alu.max)
                LNS = sp2.tile([M, 2 * SUP], F32, tag="lns", bufs=1)
                nc.scalar.activation(LNS, CLD, Act.Ln)
                nc.scalar.activation(CLD, LNS, Act.Exp, scale=-1.0)  # rcp
                IT = sp2.tile([M, 2 * SUP], F32, tag="it", bufs=1)
                nc.vector.tensor_tensor(IT, TAILB[:, 0:2 * SUP], CLD, Alu.mult)
                OMS = sp2.tile([M, SUP], F32, tag="oms", bufs=1)
                nc.vector.tensor_scalar(OMS, IT[:, SUP:2 * SUP], -1.0, 1.0,
                                        Alu.mult, Alu.add)
                TPS = sp2.tile([M, SUP], F32, tag="tps", bufs=1)
                nc.vector.scalar_tensor_tensor(TPS, TAILB[:, 2 * SUP:3 * SUP],
                                               BIG, OMS, Alu.mult, Alu.min)
                VS = sp2.tile([M, SUP], F32, tag="vs", bufs=1)
                nc.vector.scalar_tensor_tensor(VS, TPS, 0.0, IT[:, 0:SUP],
                                               Alu.max, Alu.subtract)
                WS = sp2.tile([M, SUP], F32, tag="ws", bufs=1)
                nc.vector.scalar_tensor_tensor(WS, SRS, 5.0, VS, Alu.mult,
                                               Alu.add)
                nc.vector.tensor_tensor(NF, Gm[:, ssl], WS, Alu.subtract)

                mx8 = mg.tile([M, 8], F32, tag="mx")
                nc.vector.max(mx8, NF)
                ix8 = mg.tile([M, 8], U32, tag="ix")
                nc.vector.max_index(ix8, mx8, NF)
                cmp = mg.tile([M, 1], U32, tag="cmp")
                nc.vector.tensor_tensor(cmp, mx8[:, 0:1], bv, Alu.is_gt)
                nc.vector.tensor_tensor(bv, bv, mx8[:, 0:1], Alu.max)
                ixg = mg.tile([M, 1], U32, tag="ixg")
                nc.vector.tensor_scalar(ixg, ix8[:, 0:1], s * SUP, None, Alu.add)
                nc.vector.copy_predicated(out=bi, mask=cmp, data=ixg)

        # ---------------- output ------------------------------------------
        outc = singles.tile([M, 3], I32)
        nc.vector.tensor_copy(out=outc[:, 0:1], in_=metat[:, 0:1])
        nc.vector.tensor_copy(out=outc[:, 1:2], in_=bi)
        nc.vector.tensor_copy(out=outc[:, 2:3], in_=metat[:, 1:2])
        nc.sync.dma_start(out=out, in_=outc)

    return nc


def build_nc():
    nc = bacc.Bacc("TRN2", target_bir_lowering=False, debug=False)
    t = {}
    t["cp"] = nc.dram_tensor("cp", (128, (N * C) // 128), F32, kind="ExternalInput")
    t["r1"] = nc.dram_tensor("r1", (16, N), F16, kind="ExternalInput")
    t["r2"] = nc.dram_tensor("r2", (6, N), F16, kind="ExternalInput")
    t["la"] = nc.dram_tensor("la", (128, M), F16, kind="ExternalInput")
    t["lb"] = nc.dram_tensor("lb", (128, M), F16, kind="ExternalInput")
    t["lc"] = nc.dram_tensor("lc", (36, M), F16, kind="ExternalInput")
    t["hw"] = nc.dram_tensor("hw", (M, 2), F32, kind="ExternalInput")
    t["srow"] = nc.dram_tensor("srow", (128, (N * C) // 128), F32, kind="ExternalInput")
    t["lab"] = nc.dram_tensor("lab", (M, 1), I32, kind="ExternalInput")
    t["meta"] = nc.dram_tensor("meta", (M, 2), I32, kind="ExternalInput")
    t["xtp"] = nc.dram_tensor("xtp", (C, N), F32, kind="Internal")
    t["out"] = nc.dram_tensor("out", (M, 3), I32, kind="ExternalOutput")
    emit_kernel(nc, t)
    nc.finalize()
    return nc


_NC_CACHE = None


def _split16(x):
    """fp32 -> (hi, lo) fp16 with hi + lo == x to fp32 roundoff."""
    x = np.asarray(x, np.float32)
    hi = x.astype(np.float16)
    lo = (x - hi.astype(np.float32)).astype(np.float16)
    return hi, lo


def _prep_core(cls_pred_i, loc_pred_i, cls_true_i, loc_true_i, core_idx):
    w = h = 128
    p = np.ascontiguousarray(cls_pred_i.reshape(N, C), np.float32)
    lp = (loc_pred_i.reshape(N, 4).astype(np.float32)
          / np.asarray([w, h, w, h], np.float32))
    lt = np.asarray(loc_true_i, np.float32)

    # class-major flat table input
    cp = np.ascontiguousarray(p.T.reshape(128, (N * C) // 128))

    # per-n rows
    py = [lp[:, k] for k in range(4)]          # py1, px1, py2, px2
    pa = (np.maximum(py[2] - py[0], 0.0) * np.maximum(py[3] - py[1], 0.0)
          ).astype(np.float32)
    spn = (-2.5 * lp.sum(axis=1)).astype(np.float32)
    ones = np.ones(N, np.float16)
    r1 = np.empty((16, N), np.float16)
    for k in range(4):
        hi, lo = _split16(py[k])
        r1[4 * k + 0], r1[4 * k + 1] = hi, lo
        r1[4 * k + 2], r1[4 * k + 3] = ones, ones
    pah, pal = _split16(pa)
    sph, spl = _split16(spn)
    r2 = np.stack([pah, pal, ones, ones, sph, spl])
    srow = np.ascontiguousarray(np.tile(spn, C).reshape(128, (N * C) // 128))

    # per-m weights
    ty = [lt[:, k] for k in range(4)]          # ty1, tx1, ty2, tx2
    tyh = [_split16(v) for v in ty]
    ta = (np.maximum(ty[2] - ty[0], 0.0) * np.maximum(ty[3] - ty[1], 0.0)
          ).astype(np.float32)
    tah, tal = _split16(ta)
    negones = np.full(M, -1.0, np.float16)
    onesm = np.ones(M, np.float16)
    la = np.zeros((128, M), np.float16)
    # Dy1 = ty1 - py1, Dx1 = tx1 - px1, Dy2 = ty2 - py2, Dx2 = tx2 - px2
    for g, k in enumerate([0, 1, 2, 3]):
        base = 32 * g
        la[base + 0], la[base + 1] = negones, negones
        la[base + 2], la[base + 3] = tyh[k]
    lb = np.zeros((128, M), np.float16)
    # Ky=ty2-py1 @0, Kx=tx2-px1 @32, Ly=ty1-py2 @64, Lx=tx1-px2 @96
    for base, k in [(0, 2), (32, 3), (64, 0), (96, 1)]:
        lb[base + 0], lb[base + 1] = negones, negones
        lb[base + 2], lb[base + 3] = tyh[k]
    lc = np.zeros((36, M), np.float16)
    lc[0], lc[1] = onesm, onesm
    lc[2], lc[3] = tah, tal
    lc[32], lc[33] = onesm, onesm

    hwm = np.stack([ty[2] - ty[0], ty[3] - ty[1]], axis=1).astype(np.float32)
    labels = np.argmax(cls_true_i, axis=-1).astype(np.int32)
    meta = np.stack([np.full(M, core_idx, np.int32), labels], axis=1)

    return {
        "cp": cp, "r1": r1, "r2": np.ascontiguousarray(r2),
        "la": la, "lb": lb, "lc": lc, "hw": hwm, "srow": srow,
        "lab": labels.reshape(M, 1), "meta": meta,
    }


def kernel(cls_pred, loc_pred, cls_true, loc_true, reg_mask=None):
    global _NC_CACHE
    if _NC_CACHE is None:
        _NC_CACHE = build_nc()
    nc = _NC_CACHE

    b, w, h, c = cls_pred.shape
    assert (b, w * h, c) == (B, N, C)
    in_maps = [
        _prep_core(np.asarray(cls_pred[i]), np.asarray(loc_pred[i]),
                   np.asarray(cls_true[i]), np.asarray(loc_true[i]), i)
        for i in range(B)
    ]
    res = bass_utils.run_bass_kernel_spmd(nc, in_maps, core_ids=list(range(B)))
    outs = [r["out"].reshape(M, 3) for r in res.results]
    return np.stack(outs, axis=0).astype(np.int32)


if __name__ == "__main__":
    import reference
    inputs = reference.setup_inputs()
    inputs = {k: np.asarray(v) for k, v in inputs.items()}
    got = kernel(**inputs)
    print(got[0, :5])
